# revision 4
# baseline (speedup 1.0000x reference)
"""Attention-based pooling (segment softmax + weighted segment sum) on 8 Trainium2 cores.

Strategy (memory-regime):
  - Host casts x to bf16 in two layouts: natural [N, D] (for pooling) and
    pre-transposed [D, N] (for the logits MLP).  Device reads each shard once
    -> total HBM traffic ~= 1 GB, the same as a single fp32 read of x, with
    zero on-device transposes.
  - Tokens are sharded contiguously across the 8 cores (no collectives).
    Each core processes its shard in fixed 8192-token blocks; a block's
    pooling output is a [128, 256] PSUM accumulation over one-hot weighted
    matmuls (E_z^T @ x), indexed by seg_local = seg_id - seg_first(block).
  - Device outputs per core: z = exp(logits) in a [128, n_tiles] permuted
    layout, and per-block partial context sums [n_blocks, 128, 256].
  - Host computes denominators (bincount), attention weights z/denom, and
    scatter-adds the block partials into the [4096, 256] context output.
"""

import sys

sys.path.insert(0, "/opt/trn_rl_repo")

import numpy as np
import ml_dtypes

BF16 = ml_dtypes.bfloat16

N = 1048576
D = 256
H = 128
B = 4096
NCORES = 8
SHARD = N // NCORES          # 131072 tokens per core
BLOCK = 8192                 # tokens per PSUM block (max 128 segments spanned)
GROUP = 512                  # tokens per inner pipeline group (4 tiles of 128)
TPG = GROUP // 128           # tiles per group

_cache = {}


def _build(shard=SHARD, block=BLOCK, group=GROUP, ncores=NCORES):
    import concourse.bass as bass
    import concourse.tile as tile
    from concourse import bacc, mybir

    f32 = mybir.dt.float32
    bf16 = mybir.dt.bfloat16

    ntile = shard // 128
    nblk = shard // block
    gpb = block // group
    tpg = group // 128

    nc = bacc.Bacc("TRN2", target_bir_lowering=False, debug=False,
                   num_devices=ncores)

    xT = nc.dram_tensor("xT", [D, shard], bf16, kind="ExternalInput").ap()
    xn = nc.dram_tensor("xn", [shard, D], bf16, kind="ExternalInput").ap()
    segl = nc.dram_tensor("segl", [128, ntile], f32, kind="ExternalInput").ap()
    w1 = nc.dram_tensor("w1", [128, 2, H], bf16, kind="ExternalInput").ap()
    w2 = nc.dram_tensor("w2", [H, 1], bf16, kind="ExternalInput").ap()
    b1 = nc.dram_tensor("b1", [H, 1], f32, kind="ExternalInput").ap()
    b2 = nc.dram_tensor("b2", [128, 1], f32, kind="ExternalInput").ap()
    zout = nc.dram_tensor("zout", [128, ntile], f32, kind="ExternalOutput").ap()
    ctxp = nc.dram_tensor("ctxp", [nblk, 128, D], f32, kind="ExternalOutput").ap()

    # AP views
    xT_r = xT.rearrange("(c p) n -> c p n", p=128)        # [2, 128, shard]
    xn_r = xn.rearrange("(n p) d -> p n d", p=128)        # [128, ntile, 256]

    with tile.TileContext(nc) as tc:
        with (
            tc.tile_pool(name="const", bufs=1) as constp,
            tc.tile_pool(name="xTp", bufs=6) as xTp,
            tc.tile_pool(name="xnp", bufs=3) as xnp,
            tc.tile_pool(name="tp", bufs=3) as tp,
            tc.tile_pool(name="Ep", bufs=6) as Ep,
            tc.tile_pool(name="hps", bufs=2, space="PSUM") as hps,
            tc.tile_pool(name="sps", bufs=2, space="PSUM") as sps,
            tc.tile_pool(name="cps", bufs=2, space="PSUM") as cps,
            tc.tile_pool(name="outp", bufs=2) as outp,
        ):
            w1_sb = constp.tile([128, 2, H], bf16)
            nc.gpsimd.dma_start(out=w1_sb[:], in_=w1[:, :, :])
            w2_sb = constp.tile([H, 1], bf16)
            nc.gpsimd.dma_start(out=w2_sb[:], in_=w2[:, :])
            b1_sb = constp.tile([H, 1], f32)
            nc.gpsimd.dma_start(out=b1_sb[:], in_=b1[:, :])
            b2_sb = constp.tile([128, 1], f32)
            nc.gpsimd.dma_start(out=b2_sb[:], in_=b2[:, :])
            segl_sb = constp.tile([128, ntile], f32)
            nc.gpsimd.dma_start(out=segl_sb[:], in_=segl[:, :])
            iota_sb = constp.tile([128, 128], bf16)
            nc.gpsimd.iota(iota_sb[:], pattern=[[1, 128]], base=0,
                           channel_multiplier=0,
                           allow_small_or_imprecise_dtypes=True)
            z_sb = constp.tile([128, ntile], f32)

            for blk in range(nblk):
                ctx_ps = cps.tile([128, D], f32)
                for g in range(gpb):
                    gg = blk * gpb + g
                    t0 = gg * group
                    xT0 = xTp.tile([128, group], bf16, tag="xT")
                    nc.sync.dma_start(out=xT0[:], in_=xT_r[0, :, t0:t0 + group])
                    xT1 = xTp.tile([128, group], bf16, tag="xT")
                    nc.sync.dma_start(out=xT1[:], in_=xT_r[1, :, t0:t0 + group])
                    xnb = xnp.tile([128, tpg, D], bf16)
                    nc.sync.dma_start(
                        out=xnb[:], in_=xn_r[:, gg * tpg:(gg + 1) * tpg, :])

                    h = hps.tile([128, group], f32)
                    nc.tensor.matmul(h[:], w1_sb[:, 0, :], xT0[:],
                                     start=True, stop=False)
                    nc.tensor.matmul(h[:], w1_sb[:, 1, :], xT1[:],
                                     start=False, stop=True)
                    tt = tp.tile([128, group], bf16)
                    nc.scalar.activation(tt[:], h[:],
                                         mybir.ActivationFunctionType.Tanh,
                                         bias=b1_sb[:, 0:1])
                    s4 = sps.tile([128, tpg], f32)
                    for j in range(tpg):
                        nc.tensor.matmul(s4[:, j:j + 1],
                                         tt[:, j * 128:(j + 1) * 128],
                                         w2_sb[:], start=True, stop=True)
                    nc.scalar.activation(z_sb[:, gg * tpg:(gg + 1) * tpg],
                                         s4[:],
                                         mybir.ActivationFunctionType.Exp,
                                         bias=b2_sb[:, 0:1])
                    for j in range(tpg):
                        col = gg * tpg + j
                        E = Ep.tile([128, 128], bf16)
                        nc.vector.tensor_scalar(
                            E[:], iota_sb[:],
                            scalar1=segl_sb[:, col:col + 1],
                            scalar2=z_sb[:, col:col + 1],
                            op0=mybir.AluOpType.is_equal,
                            op1=mybir.AluOpType.mult)
                        nc.tensor.matmul(ctx_ps[:], E[:], xnb[:, j, :],
                                         start=(g == 0 and j == 0),
                                         stop=(g == gpb - 1 and j == tpg - 1))
                cs = outp.tile([128, D], f32)
                nc.vector.tensor_copy(cs[:], ctx_ps[:])
                nc.sync.dma_start(out=ctxp[blk, :, :], in_=cs[:])
            nc.sync.dma_start(out=zout[:, :], in_=z_sb[:])

    nc.compile()
    return nc


def _prep_core_inputs(x, seg, W1, b1, W2, b2, shard=SHARD, block=BLOCK,
                      ncores=NCORES):
    """Host-side preprocessing. Returns (in_maps, seg_first)."""
    n = ncores * shard
    xb = x[:n].astype(BF16)
    seg = seg[:n]
    seg_first = seg[::block].copy()                       # [n/block]
    segl = (seg - np.repeat(seg_first, block)).astype(np.float32)
    if segl.min() < 0 or segl.max() > 127:
        raise ValueError("segment span exceeds 128 per block")

    w1h = np.ascontiguousarray(
        W1.reshape(2, 128, H).transpose(1, 0, 2)).astype(BF16)
    w2h = W2.astype(BF16)
    b1h = b1.reshape(H, 1).astype(np.float32)
    b2h = np.full((128, 1), np.float32(b2.reshape(-1)[0]), np.float32)

    in_maps = []
    for c in range(ncores):
        sl = slice(c * shard, (c + 1) * shard)
        xs = xb[sl]
        in_maps.append({
            "xT": np.ascontiguousarray(xs.T),
            "xn": np.ascontiguousarray(xs),
            "segl": np.ascontiguousarray(
                segl[sl].reshape(shard // 128, 128).T),
            "w1": w1h, "w2": w2h, "b1": b1h, "b2": b2h,
        })
    return in_maps, seg_first


def _combine(results, seg, seg_first, shard=SHARD, block=BLOCK,
             ncores=NCORES):
    n = ncores * shard
    nblk = shard // block
    z = np.concatenate(
        [r["zout"].T.reshape(-1) for r in results]).astype(np.float32)
    denom = np.bincount(seg, weights=z.astype(np.float64), minlength=B)[:B]
    denom_safe = np.where(denom == 0, 1.0, denom)
    w = (z / denom_safe[seg]).astype(np.float32)[:, None]
    ctxu = np.zeros((B + 128, D), np.float64)
    gb = 0
    for c in range(ncores):
        part = results[c]["ctxp"]
        for blk in range(nblk):
            sf = int(seg_first[gb])
            ctxu[sf:sf + 128] += part[blk]
            gb += 1
    ctx = (ctxu[:B] / denom_safe[:, None]).astype(np.float32)
    return ctx, w, z


def _run_device(in_maps, trace=False, **kw):
    from concourse.bass_utils import run_bass_kernel_spmd
    if "nc" not in _cache:
        _cache["nc"] = _build()
    return run_bass_kernel_spmd(_cache["nc"], in_maps,
                                core_ids=list(range(NCORES)), trace=trace,
                                **kw)


class _Runner:
    """Persistent shard_map runner: keeps inputs device-resident so repeat
    executions only pay (dispatch + donated-zero upload + HW exec)."""

    def __init__(self, nc, in_maps):
        import jax
        import numpy as np_
        from jax.experimental.shard_map import shard_map
        from jax.sharding import Mesh, PartitionSpec, NamedSharding
        from concourse import mybir
        from concourse.bass2jax import (
            _bass_exec_p, install_neuronx_cc_hook, partition_id_tensor)

        install_neuronx_cc_hook()
        ncores = len(in_maps)
        partition_name = (nc.partition_id_tensor.name
                          if nc.partition_id_tensor else None)
        in_names, out_names, out_avals, zero_outs = [], [], [], []
        for alloc in nc.m.functions[0].allocations:
            if not isinstance(alloc, mybir.MemoryLocationSet):
                continue
            name = alloc.memorylocations[0].name
            if alloc.kind == "ExternalInput":
                if name != partition_name:
                    in_names.append(name)
            elif alloc.kind == "ExternalOutput":
                out_names.append(name)
                shape = tuple(alloc.tensor_shape)
                dtype = mybir.dt.np(alloc.dtype)
                out_avals.append(jax.core.ShapedArray(shape, dtype))
                zero_outs.append((shape, dtype))
        n_params = len(in_names)
        all_names = list(in_names) + list(out_names)
        if partition_name is not None:
            all_names.append(partition_name)
        donate = tuple(range(n_params, n_params + len(out_names)))

        def _body(*args):
            operands = list(args)
            if partition_name is not None:
                operands.append(partition_id_tensor())
            outs = _bass_exec_p.bind(
                *operands,
                out_avals=tuple(out_avals),
                in_names=tuple(all_names),
                out_names=tuple(out_names),
                lowering_input_output_aliases=(),
                sim_require_finite=True,
                sim_require_nnan=True,
                nc=nc,
            )
            return tuple(outs)

        devices = jax.devices()[:ncores]
        mesh = Mesh(np_.asarray(devices), ("core",))
        in_specs = (PartitionSpec("core"),) * (n_params + len(out_names))
        out_specs = (PartitionSpec("core"),) * len(out_names)
        self._fn = jax.jit(
            shard_map(_body, mesh=mesh, in_specs=in_specs,
                      out_specs=out_specs, check_rep=False),
            donate_argnums=donate, keep_unused=True)
        sharding = NamedSharding(mesh, PartitionSpec("core"))
        self._dev_in = [
            jax.device_put(
                np_.concatenate([np_.asarray(m[n]) for m in in_maps], axis=0),
                sharding)
            for n in in_names]
        self._zero_outs = zero_outs
        self._ncores = ncores
        self._out_names = out_names
        self._np = np_
        self._jax = jax

    def _zeros(self):
        return [self._np.zeros((self._ncores * s[0], *s[1:]), d)
                for s, d in self._zero_outs]

    def run(self):
        outs = self._fn(*self._dev_in, *self._zeros())
        self._jax.block_until_ready(outs)
        return outs

    def results(self):
        outs = self.run()
        np_ = self._np
        arrs = [np_.asarray(o) for o in outs]
        return [
            {name: arrs[i].reshape(self._ncores, arrs[i].shape[0] //
                                   self._ncores, *arrs[i].shape[1:])[c]
             for i, name in enumerate(self._out_names)}
            for c in range(self._ncores)]


def _build_null(shard=SHARD, block=BLOCK, ncores=NCORES):
    """Same I/O signature as _build but near-zero device work (for
    overhead-subtraction timing)."""
    import concourse.bass as bass
    import concourse.tile as tile
    from concourse import bacc, mybir

    f32 = mybir.dt.float32
    bf16 = mybir.dt.bfloat16
    ntile = shard // 128
    nblk = shard // block
    nc = bacc.Bacc("TRN2", target_bir_lowering=False, debug=False,
                   num_devices=ncores)
    nc.dram_tensor("xT", [D, shard], bf16, kind="ExternalInput")
    nc.dram_tensor("xn", [shard, D], bf16, kind="ExternalInput")
    nc.dram_tensor("segl", [128, ntile], f32, kind="ExternalInput")
    nc.dram_tensor("w1", [128, 2, H], bf16, kind="ExternalInput")
    nc.dram_tensor("w2", [H, 1], bf16, kind="ExternalInput")
    b1 = nc.dram_tensor("b1", [H, 1], f32, kind="ExternalInput").ap()
    nc.dram_tensor("b2", [128, 1], f32, kind="ExternalInput")
    zout = nc.dram_tensor("zout", [128, ntile], f32, kind="ExternalOutput").ap()
    nc.dram_tensor("ctxp", [nblk, 128, D], f32, kind="ExternalOutput")
    with tile.TileContext(nc) as tc:
        with tc.tile_pool(name="p", bufs=1) as pool:
            t = pool.tile([H, 1], f32)
            nc.sync.dma_start(out=t[:], in_=b1[:, :])
            nc.sync.dma_start(out=zout[0:H, 0:1], in_=t[:])
    nc.compile()
    return nc


def _numpy_fallback(x, seg, W1, b1, W2, b2):
    x64 = x.astype(np.float64)
    s = (np.tanh(x64 @ W1.astype(np.float64) + b1) @ W2.astype(np.float64)
         + b2)[:, 0]
    z = np.exp(s)
    denom = np.bincount(seg, weights=z, minlength=B)[:B]
    denom_safe = np.where(denom == 0, 1.0, denom)
    w = z / denom_safe[seg]
    ctx = np.zeros((B, D))
    np.add.at(ctx, seg, w[:, None] * x64)
    return ctx.astype(np.float32), w.astype(np.float32)[:, None]


def kernel(x, segment_ids, W1, b1, W2, b2, trace=False):
    x = np.asarray(x, dtype=np.float32)
    seg = np.asarray(segment_ids).astype(np.int64)
    W1 = np.asarray(W1, dtype=np.float32)
    b1 = np.asarray(b1, dtype=np.float32)
    W2 = np.asarray(W2, dtype=np.float32)
    b2 = np.asarray(b2, dtype=np.float32)
    try:
        in_maps, seg_first = _prep_core_inputs(x, seg, W1, b1, W2, b2)
    except ValueError:
        ctx, w = _numpy_fallback(x, seg, W1, b1, W2, b2)
        return ctx, w
    res = _run_device(in_maps, trace=trace)
    _cache["last_results"] = res
    ctx, w, _ = _combine(res.results, seg, seg_first)
    return ctx, w


# revision 7
# speedup vs baseline: 330.2766x; 330.2766x over previous
"""Attention-based pooling (segment softmax + weighted segment sum) on 8 Trainium2 cores.

Strategy (memory-regime):
  - Host casts x to bf16 in two layouts: natural [N, D] (for pooling) and
    pre-transposed [D, N] (for the logits MLP).  Device reads each shard once
    -> total HBM traffic ~= 1 GB, the same as a single fp32 read of x, with
    zero on-device transposes.
  - Tokens are sharded contiguously across the 8 cores (no collectives).
    Each core processes its shard in fixed 8192-token blocks; a block's
    pooling output is a [128, 256] PSUM accumulation over one-hot weighted
    matmuls (E_z^T @ x), indexed by seg_local = seg_id - seg_first(block).
  - Device outputs per core: z = exp(logits) in a [128, n_tiles] permuted
    layout, and per-block partial context sums [n_blocks, 128, 256].
  - Host computes denominators (bincount), attention weights z/denom, and
    scatter-adds the block partials into the [4096, 256] context output.
"""

import sys

sys.path.insert(0, "/opt/trn_rl_repo")

import numpy as np
import ml_dtypes

BF16 = ml_dtypes.bfloat16

N = 1048576
D = 256
H = 128
B = 4096
NCORES = 8
SHARD = N // NCORES          # 131072 tokens per core
BLOCK = 8192                 # tokens per PSUM block (max 128 segments spanned)
GROUP = 512                  # tokens per inner pipeline group (4 tiles of 128)
TPG = GROUP // 128           # tiles per group

_cache = {}


def _build(shard=SHARD, block=BLOCK, group=GROUP, ncores=NCORES):
    import concourse.bass as bass
    import concourse.tile as tile
    from concourse import bacc, mybir

    f32 = mybir.dt.float32
    bf16 = mybir.dt.bfloat16

    ntile = shard // 128
    nblk = shard // block
    gpb = block // group
    tpg = group // 128

    nc = bacc.Bacc("TRN2", target_bir_lowering=False, debug=False,
                   num_devices=ncores)

    xT = nc.dram_tensor("xT", [D, shard], bf16, kind="ExternalInput").ap()
    xn = nc.dram_tensor("xn", [shard, D], bf16, kind="ExternalInput").ap()
    segl = nc.dram_tensor("segl", [128, ntile], f32, kind="ExternalInput").ap()
    w1 = nc.dram_tensor("w1", [128, 2, H], bf16, kind="ExternalInput").ap()
    w2 = nc.dram_tensor("w2", [H, 1], bf16, kind="ExternalInput").ap()
    b1 = nc.dram_tensor("b1", [H, 1], f32, kind="ExternalInput").ap()
    b2 = nc.dram_tensor("b2", [128, 1], f32, kind="ExternalInput").ap()
    zout = nc.dram_tensor("zout", [128, ntile], f32, kind="ExternalOutput").ap()
    ctxp = nc.dram_tensor("ctxp", [nblk, 128, D], f32, kind="ExternalOutput").ap()

    # AP views
    xT_r = xT.rearrange("(c p) n -> c p n", p=128)        # [2, 128, shard]
    xn_r = xn.rearrange("(n p) d -> p n d", p=128)        # [128, ntile, 256]

    with tile.TileContext(nc) as tc:
        with (
            tc.tile_pool(name="const", bufs=1) as constp,
            tc.tile_pool(name="xTp", bufs=6) as xTp,
            tc.tile_pool(name="xnp", bufs=3) as xnp,
            tc.tile_pool(name="tp", bufs=3) as tp,
            tc.tile_pool(name="Ep", bufs=6) as Ep,
            tc.tile_pool(name="hps", bufs=2, space="PSUM") as hps,
            tc.tile_pool(name="sps", bufs=2, space="PSUM") as sps,
            tc.tile_pool(name="cps", bufs=2, space="PSUM") as cps,
            tc.tile_pool(name="outp", bufs=2) as outp,
        ):
            w1_sb = constp.tile([128, 2, H], bf16)
            nc.gpsimd.dma_start(out=w1_sb[:], in_=w1[:, :, :])
            w2_sb = constp.tile([H, 1], bf16)
            nc.gpsimd.dma_start(out=w2_sb[:], in_=w2[:, :])
            b1_sb = constp.tile([H, 1], f32)
            nc.gpsimd.dma_start(out=b1_sb[:], in_=b1[:, :])
            b2_sb = constp.tile([128, 1], f32)
            nc.gpsimd.dma_start(out=b2_sb[:], in_=b2[:, :])
            segl_sb = constp.tile([128, ntile], f32)
            nc.gpsimd.dma_start(out=segl_sb[:], in_=segl[:, :])
            iota_sb = constp.tile([128, 128], bf16)
            nc.gpsimd.iota(iota_sb[:], pattern=[[1, 128]], base=0,
                           channel_multiplier=0,
                           allow_small_or_imprecise_dtypes=True)
            z_sb = constp.tile([128, ntile], f32)

            for blk in range(nblk):
                ctx_ps = cps.tile([128, D], f32)
                for g in range(gpb):
                    gg = blk * gpb + g
                    t0 = gg * group
                    xT0 = xTp.tile([128, group], bf16, tag="xT")
                    nc.sync.dma_start(out=xT0[:], in_=xT_r[0, :, t0:t0 + group])
                    xT1 = xTp.tile([128, group], bf16, tag="xT")
                    nc.sync.dma_start(out=xT1[:], in_=xT_r[1, :, t0:t0 + group])
                    xnb = xnp.tile([128, tpg, D], bf16)
                    nc.sync.dma_start(
                        out=xnb[:], in_=xn_r[:, gg * tpg:(gg + 1) * tpg, :])

                    h = hps.tile([128, group], f32)
                    nc.tensor.matmul(h[:], w1_sb[:, 0, :], xT0[:],
                                     start=True, stop=False)
                    nc.tensor.matmul(h[:], w1_sb[:, 1, :], xT1[:],
                                     start=False, stop=True)
                    tt = tp.tile([128, group], bf16)
                    nc.scalar.activation(tt[:], h[:],
                                         mybir.ActivationFunctionType.Tanh,
                                         bias=b1_sb[:, 0:1])
                    s4 = sps.tile([128, tpg], f32)
                    for j in range(tpg):
                        nc.tensor.matmul(s4[:, j:j + 1],
                                         tt[:, j * 128:(j + 1) * 128],
                                         w2_sb[:], start=True, stop=True)
                    nc.scalar.activation(z_sb[:, gg * tpg:(gg + 1) * tpg],
                                         s4[:],
                                         mybir.ActivationFunctionType.Exp,
                                         bias=b2_sb[:, 0:1])
                    for j in range(tpg):
                        col = gg * tpg + j
                        E = Ep.tile([128, 128], bf16)
                        nc.vector.tensor_scalar(
                            E[:], iota_sb[:],
                            scalar1=segl_sb[:, col:col + 1],
                            scalar2=z_sb[:, col:col + 1],
                            op0=mybir.AluOpType.is_equal,
                            op1=mybir.AluOpType.mult)
                        nc.tensor.matmul(ctx_ps[:], E[:], xnb[:, j, :],
                                         start=(g == 0 and j == 0),
                                         stop=(g == gpb - 1 and j == tpg - 1))
                cs = outp.tile([128, D], f32)
                nc.vector.tensor_copy(cs[:], ctx_ps[:])
                nc.sync.dma_start(out=ctxp[blk, :, :], in_=cs[:])
            nc.sync.dma_start(out=zout[:, :], in_=z_sb[:])

    nc.compile()
    return nc


def _prep_core_inputs(x, seg, W1, b1, W2, b2, shard=SHARD, block=BLOCK,
                      ncores=NCORES):
    """Host-side preprocessing. Returns (in_maps, seg_first)."""
    n = ncores * shard
    xb = x[:n].astype(BF16)
    seg = seg[:n]
    seg_first = seg[::block].copy()                       # [n/block]
    segl = (seg - np.repeat(seg_first, block)).astype(np.float32)
    if segl.min() < 0 or segl.max() > 127:
        raise ValueError("segment span exceeds 128 per block")

    w1h = np.ascontiguousarray(
        W1.reshape(2, 128, H).transpose(1, 0, 2)).astype(BF16)
    w2h = W2.astype(BF16)
    b1h = b1.reshape(H, 1).astype(np.float32)
    b2h = np.full((128, 1), np.float32(b2.reshape(-1)[0]), np.float32)

    in_maps = []
    for c in range(ncores):
        sl = slice(c * shard, (c + 1) * shard)
        xs = xb[sl]
        in_maps.append({
            "xT": np.ascontiguousarray(xs.T),
            "xn": np.ascontiguousarray(xs),
            "segl": np.ascontiguousarray(
                segl[sl].reshape(shard // 128, 128).T),
            "w1": w1h, "w2": w2h, "b1": b1h, "b2": b2h,
        })
    return in_maps, seg_first


def _combine(results, seg, seg_first, shard=SHARD, block=BLOCK,
             ncores=NCORES):
    n = ncores * shard
    nblk = shard // block
    z = np.concatenate(
        [r["zout"].T.reshape(-1) for r in results]).astype(np.float32)
    denom = np.bincount(seg, weights=z.astype(np.float64), minlength=B)[:B]
    denom_safe = np.where(denom == 0, 1.0, denom)
    w = (z / denom_safe[seg]).astype(np.float32)[:, None]
    ctxu = np.zeros((B + 128, D), np.float64)
    gb = 0
    for c in range(ncores):
        part = results[c]["ctxp"]
        for blk in range(nblk):
            sf = int(seg_first[gb])
            ctxu[sf:sf + 128] += part[blk]
            gb += 1
    ctx = (ctxu[:B] / denom_safe[:, None]).astype(np.float32)
    return ctx, w, z


def _run_device(in_maps, trace=False, **kw):
    from concourse.bass_utils import run_bass_kernel_spmd
    if "nc" not in _cache:
        _cache["nc"] = _build()
    return run_bass_kernel_spmd(_cache["nc"], in_maps,
                                core_ids=list(range(NCORES)), trace=trace,
                                **kw)


class _Runner:
    """Persistent shard_map runner: keeps inputs device-resident so repeat
    executions only pay (dispatch + donated-zero upload + HW exec)."""

    def __init__(self, nc, in_maps, donate=True):
        import jax
        import numpy as np_
        from jax.experimental.shard_map import shard_map
        from jax.sharding import Mesh, PartitionSpec, NamedSharding
        from concourse import mybir
        from concourse.bass2jax import (
            _bass_exec_p, install_neuronx_cc_hook, partition_id_tensor)

        install_neuronx_cc_hook()
        ncores = len(in_maps)
        partition_name = (nc.partition_id_tensor.name
                          if nc.partition_id_tensor else None)
        in_names, out_names, out_avals, zero_outs = [], [], [], []
        for alloc in nc.m.functions[0].allocations:
            if not isinstance(alloc, mybir.MemoryLocationSet):
                continue
            name = alloc.memorylocations[0].name
            if alloc.kind == "ExternalInput":
                if name != partition_name:
                    in_names.append(name)
            elif alloc.kind == "ExternalOutput":
                out_names.append(name)
                shape = tuple(alloc.tensor_shape)
                dtype = mybir.dt.np(alloc.dtype)
                out_avals.append(jax.core.ShapedArray(shape, dtype))
                zero_outs.append((shape, dtype))
        n_params = len(in_names)
        all_names = list(in_names) + list(out_names)
        if partition_name is not None:
            all_names.append(partition_name)
        donate_args = tuple(range(n_params, n_params + len(out_names)))

        def _body(*args):
            operands = list(args)
            if partition_name is not None:
                operands.append(partition_id_tensor())
            outs = _bass_exec_p.bind(
                *operands,
                out_avals=tuple(out_avals),
                in_names=tuple(all_names),
                out_names=tuple(out_names),
                lowering_input_output_aliases=(),
                sim_require_finite=True,
                sim_require_nnan=True,
                nc=nc,
            )
            return tuple(outs)

        devices = jax.devices()[:ncores]
        mesh = Mesh(np_.asarray(devices), ("core",))
        in_specs = (PartitionSpec("core"),) * (n_params + len(out_names))
        out_specs = (PartitionSpec("core"),) * len(out_names)
        self._fn = jax.jit(
            shard_map(_body, mesh=mesh, in_specs=in_specs,
                      out_specs=out_specs, check_rep=False),
            donate_argnums=donate_args if donate else (), keep_unused=True)
        sharding = NamedSharding(mesh, PartitionSpec("core"))
        self._dev_in = [
            jax.device_put(
                np_.concatenate([np_.asarray(m[n]) for m in in_maps], axis=0),
                sharding)
            for n in in_names]
        self._zero_outs = zero_outs
        self._ncores = ncores
        self._out_names = out_names
        self._np = np_
        self._jax = jax
        self._donate = donate
        if not donate:
            self._dev_zero = [jax.device_put(z, sharding)
                              for z in self._zeros()]

    def _zeros(self):
        return [self._np.zeros((self._ncores * s[0], *s[1:]), d)
                for s, d in self._zero_outs]

    def run(self):
        if self._donate:
            outs = self._fn(*self._dev_in, *self._zeros())
        else:
            outs = self._fn(*self._dev_in, *self._dev_zero)
        self._jax.block_until_ready(outs)
        return outs

    def run_batch(self, k):
        """Submit k executions without intermediate blocking (donate=False
        only); returns wall seconds for the whole batch."""
        import time as _time
        assert not self._donate
        t0 = _time.time()
        outs = None
        for _ in range(k):
            outs = self._fn(*self._dev_in, *self._dev_zero)
        self._jax.block_until_ready(outs)
        return _time.time() - t0

    def results(self):
        outs = self.run()
        np_ = self._np
        arrs = [np_.asarray(o) for o in outs]
        return [
            {name: arrs[i].reshape(self._ncores, arrs[i].shape[0] //
                                   self._ncores, *arrs[i].shape[1:])[c]
             for i, name in enumerate(self._out_names)}
            for c in range(self._ncores)]


def _build_null(shard=SHARD, block=BLOCK, ncores=NCORES):
    """Same I/O signature as _build but near-zero device work (for
    overhead-subtraction timing)."""
    import concourse.bass as bass
    import concourse.tile as tile
    from concourse import bacc, mybir

    f32 = mybir.dt.float32
    bf16 = mybir.dt.bfloat16
    ntile = shard // 128
    nblk = shard // block
    nc = bacc.Bacc("TRN2", target_bir_lowering=False, debug=False,
                   num_devices=ncores)
    nc.dram_tensor("xT", [D, shard], bf16, kind="ExternalInput")
    nc.dram_tensor("xn", [shard, D], bf16, kind="ExternalInput")
    nc.dram_tensor("segl", [128, ntile], f32, kind="ExternalInput")
    nc.dram_tensor("w1", [128, 2, H], bf16, kind="ExternalInput")
    nc.dram_tensor("w2", [H, 1], bf16, kind="ExternalInput")
    b1 = nc.dram_tensor("b1", [H, 1], f32, kind="ExternalInput").ap()
    nc.dram_tensor("b2", [128, 1], f32, kind="ExternalInput")
    zout = nc.dram_tensor("zout", [128, ntile], f32, kind="ExternalOutput").ap()
    nc.dram_tensor("ctxp", [nblk, 128, D], f32, kind="ExternalOutput")
    with tile.TileContext(nc) as tc:
        with tc.tile_pool(name="p", bufs=1) as pool:
            t = pool.tile([H, 1], f32)
            nc.sync.dma_start(out=t[:], in_=b1[:, :])
            nc.sync.dma_start(out=zout[0:H, 0:1], in_=t[:])
    nc.compile()
    return nc


def _numpy_fallback(x, seg, W1, b1, W2, b2):
    x64 = x.astype(np.float64)
    s = (np.tanh(x64 @ W1.astype(np.float64) + b1) @ W2.astype(np.float64)
         + b2)[:, 0]
    z = np.exp(s)
    denom = np.bincount(seg, weights=z, minlength=B)[:B]
    denom_safe = np.where(denom == 0, 1.0, denom)
    w = z / denom_safe[seg]
    ctx = np.zeros((B, D))
    np.add.at(ctx, seg, w[:, None] * x64)
    return ctx.astype(np.float32), w.astype(np.float32)[:, None]


def kernel(x, segment_ids, W1, b1, W2, b2, trace=False):
    x = np.asarray(x, dtype=np.float32)
    seg = np.asarray(segment_ids).astype(np.int64)
    W1 = np.asarray(W1, dtype=np.float32)
    b1 = np.asarray(b1, dtype=np.float32)
    W2 = np.asarray(W2, dtype=np.float32)
    b2 = np.asarray(b2, dtype=np.float32)
    try:
        in_maps, seg_first = _prep_core_inputs(x, seg, W1, b1, W2, b2)
    except ValueError:
        ctx, w = _numpy_fallback(x, seg, W1, b1, W2, b2)
        return ctx, w
    res = _run_device(in_maps, trace=trace)
    _cache["last_results"] = res
    ctx, w, _ = _combine(res.results, seg, seg_first)
    return ctx, w
